# revision 1
# baseline (speedup 1.0000x reference)
"""3-layer LSTM (B=256,T=512,I=256,H=512) + linear head on 8 NeuronCores.

Strategy: data-parallel over batch (32/core). Per layer, the input-side
matmul G = Wih @ x_t (+ biases) for a *chunk* of future time steps is
computed at full PE efficiency (N=512 streams) and interleaved with the
sequential h-recurrence of the current chunk; G never leaves SBUF.
Gate layout: gates.T packed [128 part, 512 cols] = 16 slots of 32 batch
cols in slot order i|f|o|g, all in ONE PSUM bank per step, preloaded
with G via one DVE copy, accumulated by 64 weight-stationary bf16
matmuls (K=128, M=128, N=32), then 2 ACT instrs (sigmoid over i|f|o,
tanh over g) evacuate to SBUF. c stays fp32-resident; h is written
bf16 directly into the layout the next matmul and the next layer's
batched input matmul consume.
"""

import numpy as np
import ml_dtypes
from contextlib import ExitStack

import concourse.bass as bass
import concourse.bacc as bacc
import concourse.tile as tile
from concourse import mybir
from concourse.bass_utils import run_bass_kernel_spmd

BF16 = mybir.dt.bfloat16
F32 = mybir.dt.float32
AF = mybir.ActivationFunctionType

B, T, I, H, O = 256, 512, 256, 512, 3
NCORES = 8
BL = B // NCORES          # 32 batch rows per core
SC = 16                   # time steps per chunk
CW = SC * BL              # 512 cols per chunk
NCH = T // SC             # 32 chunks
TOT = T * BL              # 16384 cols total
SLACK = 2 * CW            # prefetch overrun slack (cols)

# gate blocks in psum-slot order: i | f | o | g  (slot = blk*4 + j)
# block -> base row in the canonical (i,f,g,o) 2048 gate layout
GATE_BASE = [0, 512, 1536, 1024]   # i, f, o, g
KCS = [2, 4, 4]                    # K chunks per layer (256, 512, 512)


def _slot_row(slot):
    return GATE_BASE[slot // 4] + 128 * (slot % 4)


def _build():
    nc = bacc.Bacc("TRN2", target_bir_lowering=False, debug=False,
                   num_devices=NCORES)

    xt = nc.dram_tensor("x_t", (128, 2, TOT + SLACK), BF16, kind="ExternalInput")
    wih = [nc.dram_tensor(f"wih{l}", (128, KCS[l] * 2048), BF16,
                          kind="ExternalInput") for l in range(3)]
    whh = [nc.dram_tensor(f"whh{l}", (128, 4 * 2048), BF16,
                          kind="ExternalInput") for l in range(3)]
    bias_d = nc.dram_tensor("bias", (128, 48), F32, kind="ExternalInput")
    fcw_d = nc.dram_tensor("fcw", (128, 12), BF16, kind="ExternalInput")
    fcb_d = nc.dram_tensor("fcb", (3, 1), F32, kind="ExternalInput")
    out_d = nc.dram_tensor("out", (3, BL), F32, kind="ExternalOutput")

    with tile.TileContext(nc) as tc, ExitStack() as ctx:
        dram = ctx.enter_context(tc.tile_pool(name="dram", bufs=1, space="DRAM"))
        hdr = dram.tile([128, 4, TOT + SLACK], BF16)   # inter-layer H seq

        const = ctx.enter_context(tc.tile_pool(name="const", bufs=1))
        wih_sb = [const.tile([128, KCS[l] * 2048], BF16, tag=f"wih{l}",
                             name=f"wih_sb{l}") for l in range(3)]
        whh_sb = [const.tile([128, 4 * 2048], BF16, tag=f"whh{l}",
                             name=f"whh_sb{l}") for l in range(3)]
        bias_sb = const.tile([128, 48], F32, tag="bias")
        fcw_sb = const.tile([128, 12], BF16, tag="fcw")
        fcb_sb = const.tile([3, 1], F32, tag="fcb")
        for l in range(3):
            nc.sync.dma_start(wih_sb[l][:], wih[l].ap())
            nc.sync.dma_start(whh_sb[l][:], whh[l].ap())
        nc.sync.dma_start(bias_sb[:], bias_d.ap())
        nc.sync.dma_start(fcw_sb[:], fcw_d.ap())
        nc.sync.dma_start(fcb_sb[:], fcb_d.ap())

        big = ctx.enter_context(tc.tile_pool(name="big", bufs=1))
        g_buf = big.tile([128, 2 * 16 * CW], BF16, tag="gbuf")     # 4MB
        in_buf = big.tile([128, 4 * 4 * CW], BF16, tag="inbuf")    # 2MB
        h_stage = big.tile([128, 2 * 4 * CW], BF16, tag="hstage")  # 1MB
        c_t = big.tile([128, 128], F32, tag="cstate")

        g3 = g_buf[:].rearrange("p (s c) -> p s c", c=CW)    # [128, 32, CW]
        i3 = in_buf[:].rearrange("p (b c) -> p b c", c=CW)   # [128, 16, CW]
        h3 = h_stage[:].rearrange("p (x c) -> p x c", c=CW)  # [128, 8, CW]

        ew = ctx.enter_context(tc.tile_pool(name="ew", bufs=2))
        ps_rec = ctx.enter_context(tc.tile_pool(name="psr", bufs=2, space="PSUM"))
        ps_pa = ctx.enter_context(tc.tile_pool(name="psa", bufs=2, space="PSUM"))
        ps_fc = ctx.enter_context(tc.tile_pool(name="psf", bufs=1, space="PSUM"))

        def phase_a_slot(l, slot, in_base, g_base, in_ap):
            """G[slot] for one chunk: Kc matmuls (N=CW) + bias ACT."""
            kc = KCS[l]
            ps = ps_pa.tile([128, CW], F32, tag="pa")
            for k in range(kc):
                nc.tensor.matmul(
                    ps[:],
                    lhsT=wih_sb[l][:, k * 2048 + _slot_row(slot):
                                   k * 2048 + _slot_row(slot) + 128],
                    rhs=in_ap(in_base + k),
                    start=(k == 0), stop=(k == kc - 1),
                )
            nc.scalar.activation(
                g3[:, bass.ds(g_base + slot, 1), :].rearrange("p a c -> p (a c)"),
                ps[:], AF.Identity, bias=bias_sb[:, l * 16 + slot: l * 16 + slot + 1])

        def rec_step(l, s, g_base, h_rd, h_wr, pa_emit):
            """One recurrence time step; h_rd/h_wr are h3 block bases."""
            ps = ps_rec.tile([128, 512], F32, tag="rec")
            nc.vector.tensor_copy(
                ps[:].rearrange("p (a b) -> p a b", b=BL),
                g3[:, bass.ds(g_base, 16), s * BL: (s + 1) * BL])
            # h[t-1]: last slot of the other-parity buffer for s=0,
            # else slot s-1 of the current chunk's buffer
            hp_base = h_rd if s == 0 else h_wr
            hp_col = ((SC - 1) if s == 0 else (s - 1)) * BL
            for slot in range(16):
                for k in range(4):
                    nc.tensor.matmul(
                        ps[:, slot * BL:(slot + 1) * BL],
                        lhsT=whh_sb[l][:, k * 2048 + _slot_row(slot):
                                       k * 2048 + _slot_row(slot) + 128],
                        rhs=h3[:, bass.ds(hp_base + k, 1),
                               hp_col:hp_col + BL].rearrange("p a c -> p (a c)"),
                        start=False, stop=(k == 3), skip_group_check=True,
                    )
            gt = ew.tile([128, 512], F32, tag="gates")
            nc.scalar.activation(gt[:, 0:384], ps[:, 0:384], AF.Sigmoid)
            nc.scalar.activation(gt[:, 384:512], ps[:, 384:512], AF.Tanh)
            t1 = ew.tile([128, 128], F32, tag="t1")
            t2 = ew.tile([128, 128], F32, tag="t2")
            nc.vector.tensor_mul(t1[:], gt[:, 0:128], gt[:, 384:512])    # i*g
            nc.vector.tensor_mul(t2[:], gt[:, 128:256], c_t[:])          # f*c
            nc.vector.tensor_add(c_t[:], t1[:], t2[:])
            th = ew.tile([128, 128], F32, tag="th")
            nc.scalar.activation(th[:], c_t[:], AF.Tanh)
            nc.vector.tensor_mul(
                h3[:, bass.ds(h_wr, 4), s * BL:(s + 1) * BL],
                gt[:, 256:384].rearrange("p (a b) -> p a b", b=BL),
                th[:].rearrange("p (a b) -> p a b", b=BL))
            if pa_emit is not None:
                pa_emit(s)

        for l in range(3):
            in_dram = xt.ap() if l == 0 else hdr[:]
            kc = KCS[l]

            # prologue: In chunks 0,1 -> bufs 0,1 ; G chunk 0 -> parity 0
            nc.sync.dma_start(i3[:, 0:kc, :], in_dram[:, :, 0:CW])
            nc.sync.dma_start(i3[:, kc:2 * kc, :], in_dram[:, :, CW:2 * CW])
            for slot in range(16):
                phase_a_slot(l, slot, 0, 0,
                             lambda idx: i3[:, bass.ds(idx, 1), :]
                             .rearrange("p a c -> p (a c)"))
            nc.vector.memset(c_t[:], 0.0)
            nc.vector.memset(h3[:, bass.ds(4, 4), (SC - 1) * BL: SC * BL], 0.0)

            def body(iv, l=l, kc=kc, in_dram=in_dram):
                p2 = iv & 1
                q2 = (iv + 1) & 1
                ld_buf = ((iv + 2) & 3) * kc
                use_buf = ((iv + 1) & 3) * kc
                nc.sync.dma_start(
                    i3[:, bass.ds(ld_buf, kc), :],
                    in_dram[:, :, bass.ds((iv + 2) * CW, CW)])

                def pa_emit(s, l=l, use_buf=use_buf, q2=q2):
                    phase_a_slot(l, s, use_buf, q2 * 16,
                                 lambda idx: i3[:, bass.ds(idx, 1), :]
                                 .rearrange("p a c -> p (a c)"))

                for s in range(SC):
                    rec_step(l, s, p2 * 16, q2 * 4, p2 * 4, pa_emit)
                if l < 2:
                    nc.sync.dma_start(
                        hdr[:, :, bass.ds(iv * CW, CW)],
                        h3[:, bass.ds(p2 * 4, 4), :])

            with tc.For_i(0, NCH, 1) as iv:
                body(iv)

        # final linear head: out.T [3, BL] = fcW @ h_last (+ fcB)
        hb = ((NCH - 1) & 1) * 4
        ps = ps_fc.tile([3, BL], F32, tag="fc")
        for k in range(4):
            nc.tensor.matmul(
                ps[:], lhsT=fcw_sb[:, k * 3:(k + 1) * 3],
                rhs=h3[:, bass.ds(hb + k, 1), (SC - 1) * BL: SC * BL]
                .rearrange("p a c -> p (a c)"),
                start=(k == 0), stop=(k == 3))
        ob = ew.tile([3, BL], F32, tag="out")
        nc.scalar.activation(ob[:], ps[:], AF.Identity, bias=fcb_sb[:])
        nc.sync.dma_start(out_d.ap(), ob[:])

    nc.compile()
    return nc


def _prep(inputs):
    """Host-side layout prep. Returns per-core in_maps."""
    bf = ml_dtypes.bfloat16
    x = np.asarray(inputs["x"], np.float32)
    wihs = [np.asarray(inputs[f"Wih{l}"], np.float32) for l in range(3)]
    whhs = [np.asarray(inputs[f"Whh{l}"], np.float32) for l in range(3)]

    def wt_pack(w, kcs):  # [2048, K] -> [128, kcs*2048]
        return np.ascontiguousarray(
            w.T.reshape(kcs, 128, 2048).transpose(1, 0, 2)
            .reshape(128, kcs * 2048)).astype(bf)

    shared = {}
    for l in range(3):
        shared[f"wih{l}"] = wt_pack(wihs[l], KCS[l])
        shared[f"whh{l}"] = wt_pack(whhs[l], 4)
    bias = np.zeros((128, 48), np.float32)
    for l in range(3):
        bl_ = (np.asarray(inputs[f"bih{l}"], np.float32)
               + np.asarray(inputs[f"bhh{l}"], np.float32))
        for slot in range(16):
            r = _slot_row(slot)
            bias[:, l * 16 + slot] = bl_[r:r + 128]
    shared["bias"] = bias
    shared["fcw"] = np.ascontiguousarray(
        np.asarray(inputs["fcW"], np.float32).T.reshape(4, 128, 3)
        .transpose(1, 0, 2).reshape(128, 12)).astype(bf)
    shared["fcb"] = np.asarray(inputs["fcB"], np.float32).reshape(3, 1)

    in_maps = []
    for c in range(NCORES):
        xc = x[c * BL:(c + 1) * BL]                       # [32, 512, 256]
        xp = xc.transpose(2, 1, 0).reshape(2, 128, TOT)   # [2,128,16384]
        xp = np.ascontiguousarray(xp.transpose(1, 0, 2))  # [128,2,16384]
        xp = np.concatenate(
            [xp, np.zeros((128, 2, SLACK), np.float32)], axis=2).astype(bf)
        in_maps.append({"x_t": xp, **shared})
    return in_maps


_NC_CACHE = None


def kernel(**inputs):
    global _NC_CACHE
    if _NC_CACHE is None:
        _NC_CACHE = _build()
    nc = _NC_CACHE
    in_maps = _prep(inputs)
    res = run_bass_kernel_spmd(nc, in_maps, core_ids=list(range(NCORES)))
    out = np.empty((B, O), np.float32)
    for c in range(NCORES):
        out[c * BL:(c + 1) * BL] = res.results[c]["out"].T
    return out



# revision 7
# speedup vs baseline: 4.1523x; 4.1523x over previous
"""3-layer LSTM (B=256,T=512,I=256,H=512) + linear head on 8 NeuronCores.

Strategy: data-parallel over batch (32/core). Per layer, the input-side
matmul G = Wih @ x_t (+ biases) for a *chunk* of future time steps is
computed at full PE efficiency (N=512 streams) and interleaved with the
sequential h-recurrence of the current chunk; G never leaves SBUF.
Gate layout: gates.T packed [128 part, 512 cols] = 16 slots of 32 batch
cols in slot order i|f|o|g, all in ONE PSUM bank per step, preloaded
with G via one DVE copy, accumulated by 64 weight-stationary bf16
matmuls (K=128, M=128, N=32), then 2 ACT instrs (sigmoid over i|f|o,
tanh over g) evacuate to SBUF. c stays fp32-resident; h is written
bf16 directly into the layout the next matmul and the next layer's
batched input matmul consume.
"""

import numpy as np
import ml_dtypes
from contextlib import ExitStack

import concourse.bass as bass
import concourse.bacc as bacc
import concourse.tile as tile
from concourse import mybir
from concourse.bass_utils import run_bass_kernel_spmd

BF16 = mybir.dt.bfloat16
F32 = mybir.dt.float32
AF = mybir.ActivationFunctionType

B, T, I, H, O = 256, 512, 256, 512, 3
NCORES = 8
BL = B // NCORES          # 32 batch rows per core
SC = 16                   # time steps per chunk
CW = SC * BL              # 512 cols per chunk
NCH = T // SC             # 32 chunks
TOT = T * BL              # 16384 cols total
SLACK = 2 * CW            # prefetch overrun slack (cols)

# gate blocks in psum-slot order: i | f | o | g  (slot = blk*4 + j)
# block -> base row in the canonical (i,f,g,o) 2048 gate layout
GATE_BASE = [0, 512, 1536, 1024]   # i, f, o, g
KCS = [2, 4, 4]                    # K chunks per layer (256, 512, 512)


def _slot_row(slot):
    return GATE_BASE[slot // 4] + 128 * (slot % 4)


def _build():
    nc = bacc.Bacc("TRN2", target_bir_lowering=False, debug=False,
                   num_devices=NCORES)

    xt = nc.dram_tensor("x_t", (128, 2, TOT + SLACK), BF16, kind="ExternalInput")
    wih = [nc.dram_tensor(f"wih{l}", (128, KCS[l] * 2048), BF16,
                          kind="ExternalInput") for l in range(3)]
    whh = [nc.dram_tensor(f"whh{l}", (128, 4 * 2048), BF16,
                          kind="ExternalInput") for l in range(3)]
    bias_d = nc.dram_tensor("bias", (128, 48), F32, kind="ExternalInput")
    fcw_d = nc.dram_tensor("fcw", (128, 12), BF16, kind="ExternalInput")
    fcb_d = nc.dram_tensor("fcb", (3, 1), F32, kind="ExternalInput")
    out_d = nc.dram_tensor("out", (3, BL), F32, kind="ExternalOutput")

    with tile.TileContext(nc) as tc, ExitStack() as ctx:
        dram = ctx.enter_context(tc.tile_pool(name="dram", bufs=1, space="DRAM"))
        hdr = dram.tile([128, 4, TOT + SLACK], BF16)   # inter-layer H seq

        const = ctx.enter_context(tc.tile_pool(name="const", bufs=1))
        wih_sb = [const.tile([128, KCS[l] * 2048], BF16, tag=f"wih{l}",
                             name=f"wih_sb{l}") for l in range(3)]
        whh_sb = [const.tile([128, 4 * 2048], BF16, tag=f"whh{l}",
                             name=f"whh_sb{l}") for l in range(3)]
        bias_sb = const.tile([128, 48], F32, tag="bias")
        fcw_sb = const.tile([128, 12], BF16, tag="fcw")
        fcb_sb = const.tile([3, 1], F32, tag="fcb")
        for l in range(3):
            nc.sync.dma_start(wih_sb[l][:], wih[l].ap())
            nc.sync.dma_start(whh_sb[l][:], whh[l].ap())
        nc.sync.dma_start(bias_sb[:], bias_d.ap())
        nc.sync.dma_start(fcw_sb[:], fcw_d.ap())
        nc.sync.dma_start(fcb_sb[:], fcb_d.ap())

        big = ctx.enter_context(tc.tile_pool(name="big", bufs=1))
        g_buf = big.tile([128, 2 * 16 * CW], BF16, tag="gbuf")     # 4MB
        in_buf = big.tile([128, 4 * 4 * CW], BF16, tag="inbuf")    # 2MB
        h_stage = big.tile([128, 2 * 4 * CW], BF16, tag="hstage")  # 1MB
        c_t = big.tile([128, 128], F32, tag="cstate")

        g3 = g_buf[:].rearrange("p (s c) -> p s c", c=CW)    # [128, 32, CW]
        i3 = in_buf[:].rearrange("p (b c) -> p b c", c=CW)   # [128, 16, CW]
        h3 = h_stage[:].rearrange("p (x c) -> p x c", c=CW)  # [128, 8, CW]

        ew = ctx.enter_context(tc.tile_pool(name="ew", bufs=2))
        ps_rec = ctx.enter_context(tc.tile_pool(name="psr", bufs=4, space="PSUM"))
        ps_pa = ctx.enter_context(tc.tile_pool(name="psa", bufs=2, space="PSUM"))
        ps_fc = ctx.enter_context(tc.tile_pool(name="psf", bufs=1, space="PSUM"))

        def phase_a_slot(l, slot, in_base, g_base, in_ap):
            """G[slot] for one chunk: Kc matmuls (N=CW) + bias ACT."""
            kc = KCS[l]
            ps = ps_pa.tile([128, CW], F32, tag="pa")
            for k in range(kc):
                nc.tensor.matmul(
                    ps[:],
                    lhsT=wih_sb[l][:, k * 2048 + _slot_row(slot):
                                   k * 2048 + _slot_row(slot) + 128],
                    rhs=in_ap(in_base + k),
                    start=(k == 0), stop=(k == kc - 1),
                )
            nc.scalar.activation(
                g3[:, bass.ds(g_base + slot, 1), :].rearrange("p a c -> p (a c)"),
                ps[:], AF.Identity, bias=bias_sb[:, l * 16 + slot: l * 16 + slot + 1])

        def rec_step(l, s, g_base, h_rd, h_wr, pa_emit):
            """One recurrence time step; h_rd/h_wr are h3 block bases."""
            ps = ps_rec.tile([128, 512], F32, tag="rec")
            nc.vector.tensor_copy(
                ps[:].rearrange("p (a b) -> p a b", b=BL),
                g3[:, bass.ds(g_base, 16), s * BL: (s + 1) * BL])
            # h[t-1]: last slot of the other-parity buffer for s=0,
            # else slot s-1 of the current chunk's buffer
            hp_base = h_rd if s == 0 else h_wr
            hp_col = ((SC - 1) if s == 0 else (s - 1)) * BL
            for slot in range(16):
                for k in range(4):
                    nc.tensor.matmul(
                        ps[:, slot * BL:(slot + 1) * BL],
                        lhsT=whh_sb[l][:, k * 2048 + _slot_row(slot):
                                       k * 2048 + _slot_row(slot) + 128],
                        rhs=h3[:, bass.ds(hp_base + k, 1),
                               hp_col:hp_col + BL].rearrange("p a c -> p (a c)"),
                        start=False, stop=(k == 3), skip_group_check=True,
                    )
            gt = ew.tile([128, 512], F32, tag="gates")
            nc.scalar.activation(gt[:, 0:384], ps[:, 0:384], AF.Sigmoid)
            nc.scalar.activation(gt[:, 384:512], ps[:, 384:512], AF.Tanh)
            t1 = ew.tile([128, 128], F32, tag="t1")
            t2 = ew.tile([128, 128], F32, tag="t2")
            nc.vector.tensor_mul(t1[:], gt[:, 0:128], gt[:, 384:512])    # i*g
            nc.vector.tensor_mul(t2[:], gt[:, 128:256], c_t[:])          # f*c
            nc.vector.tensor_add(c_t[:], t1[:], t2[:])
            th = ew.tile([128, 128], F32, tag="th")
            nc.scalar.activation(th[:], c_t[:], AF.Tanh)
            nc.vector.tensor_mul(
                h3[:, bass.ds(h_wr, 4), s * BL:(s + 1) * BL],
                gt[:, 256:384].rearrange("p (a b) -> p a b", b=BL),
                th[:].rearrange("p (a b) -> p a b", b=BL))
            if pa_emit is not None:
                pa_emit(s)

        for l in range(3):
            in_dram = xt.ap() if l == 0 else hdr[:]
            kc = KCS[l]

            # prologue: In chunks 0,1 -> bufs 0,1 ; G chunk 0 -> parity 0
            nc.sync.dma_start(i3[:, 0:kc, :], in_dram[:, :, 0:CW])
            nc.sync.dma_start(i3[:, kc:2 * kc, :], in_dram[:, :, CW:2 * CW])
            for slot in range(16):
                phase_a_slot(l, slot, 0, 0,
                             lambda idx: i3[:, bass.ds(idx, 1), :]
                             .rearrange("p a c -> p (a c)"))
            nc.vector.memset(c_t[:], 0.0)
            nc.vector.memset(h3[:, bass.ds(4, 4), (SC - 1) * BL: SC * BL], 0.0)

            def body(u, iv4, l=l, kc=kc, in_dram=in_dram):
                # u: python-int chunk index mod 4 -> ALL SBUF addressing is
                # static; only DMA HBM offsets depend on the loop variable.
                p2 = u & 1
                q2 = (u + 1) & 1
                ld_buf = ((u + 2) & 3) * kc
                use_buf = ((u + 1) & 3) * kc
                nc.sync.dma_start(
                    i3[:, bass.ds(ld_buf, kc), :],
                    in_dram[:, :, bass.ds(iv4 * (4 * CW) + (u + 2) * CW, CW)])

                def pa_emit(s, l=l, use_buf=use_buf, q2=q2):
                    phase_a_slot(l, s, use_buf, q2 * 16,
                                 lambda idx: i3[:, bass.ds(idx, 1), :]
                                 .rearrange("p a c -> p (a c)"))

                for s in range(SC):
                    rec_step(l, s, p2 * 16, q2 * 4, p2 * 4, pa_emit)
                if l < 2:
                    nc.sync.dma_start(
                        hdr[:, :, bass.ds(iv4 * (4 * CW) + u * CW, CW)],
                        h3[:, bass.ds(p2 * 4, 4), :])

            with tc.For_i(0, NCH // 4, 1) as iv4:
                for u in range(4):
                    body(u, iv4)

        # final linear head: out.T [3, BL] = fcW @ h_last (+ fcB)
        hb = ((NCH - 1) & 1) * 4
        ps = ps_fc.tile([3, BL], F32, tag="fc")
        for k in range(4):
            nc.tensor.matmul(
                ps[:], lhsT=fcw_sb[:, k * 3:(k + 1) * 3],
                rhs=h3[:, bass.ds(hb + k, 1), (SC - 1) * BL: SC * BL]
                .rearrange("p a c -> p (a c)"),
                start=(k == 0), stop=(k == 3))
        ob = ew.tile([3, BL], F32, tag="out")
        nc.scalar.activation(ob[:], ps[:], AF.Identity, bias=fcb_sb[:])
        nc.sync.dma_start(out_d.ap(), ob[:])

    nc.compile()
    return nc


def _prep(inputs):
    """Host-side layout prep. Returns per-core in_maps."""
    bf = ml_dtypes.bfloat16
    x = np.asarray(inputs["x"], np.float32)
    wihs = [np.asarray(inputs[f"Wih{l}"], np.float32) for l in range(3)]
    whhs = [np.asarray(inputs[f"Whh{l}"], np.float32) for l in range(3)]

    def wt_pack(w, kcs):  # [2048, K] -> [128, kcs*2048]
        return np.ascontiguousarray(
            w.T.reshape(kcs, 128, 2048).transpose(1, 0, 2)
            .reshape(128, kcs * 2048)).astype(bf)

    shared = {}
    for l in range(3):
        shared[f"wih{l}"] = wt_pack(wihs[l], KCS[l])
        shared[f"whh{l}"] = wt_pack(whhs[l], 4)
    bias = np.zeros((128, 48), np.float32)
    for l in range(3):
        bl_ = (np.asarray(inputs[f"bih{l}"], np.float32)
               + np.asarray(inputs[f"bhh{l}"], np.float32))
        for slot in range(16):
            r = _slot_row(slot)
            bias[:, l * 16 + slot] = bl_[r:r + 128]
    shared["bias"] = bias
    shared["fcw"] = np.ascontiguousarray(
        np.asarray(inputs["fcW"], np.float32).T.reshape(4, 128, 3)
        .transpose(1, 0, 2).reshape(128, 12)).astype(bf)
    shared["fcb"] = np.asarray(inputs["fcB"], np.float32).reshape(3, 1)

    in_maps = []
    for c in range(NCORES):
        xc = x[c * BL:(c + 1) * BL]                       # [32, 512, 256]
        xp = xc.transpose(2, 1, 0).reshape(2, 128, TOT)   # [2,128,16384]
        xp = np.ascontiguousarray(xp.transpose(1, 0, 2))  # [128,2,16384]
        xp = np.concatenate(
            [xp, np.zeros((128, 2, SLACK), np.float32)], axis=2).astype(bf)
        in_maps.append({"x_t": xp, **shared})
    return in_maps


_NC_CACHE = None


def kernel(**inputs):
    global _NC_CACHE
    if _NC_CACHE is None:
        _NC_CACHE = _build()
    nc = _NC_CACHE
    in_maps = _prep(inputs)
    res = run_bass_kernel_spmd(nc, in_maps, core_ids=list(range(NCORES)))
    out = np.empty((B, O), np.float32)
    for c in range(NCORES):
        out[c * BL:(c + 1) * BL] = res.results[c]["out"].T
    return out



# revision 9
# speedup vs baseline: 9.0106x; 2.1700x over previous
"""3-layer LSTM (B=256,T=512,I=256,H=512) + linear head on 8 NeuronCores.

V4: data-parallel over batch (32/core), gate-major weight-stationary
recurrence (64 LDW+MM pairs per layer-step at ~27ns/pair warm), with the
THREE LAYERS' recurrences interleaved in a chunk-lagged software pipeline
(layer l runs chunk c-2l) so each layer's activation/elementwise tail is
hidden under the other layers' matmul streams.  Inter-layer h stays in
SBUF (no DRAM roundtrip).  All SBUF addressing is static: the For_i body
unrolls 4 macro-steps (only DMA HBM offsets use the loop variable), which
keeps the Tensor sequencer free of per-instruction address regops.

Per layer-step: PSUM [128,512] = 16 gate slots x 32 batch (slot order
i|f|o|g), preloaded with G = Wih@x(+bias) via one DVE cast, accumulated
by 64 weight-stationary bf16 matmuls (K=128,M=128,N=32), evacuated by 2
ACTs (sigmoid|tanh); c stays f32-resident; h is written bf16 into the
layout both the next matmul and the next layer's phase-A consume.
"""

import numpy as np
import ml_dtypes
from contextlib import ExitStack

import concourse.bass as bass
import concourse.bacc as bacc
import concourse.tile as tile
from concourse import mybir
from concourse.bass_utils import run_bass_kernel_spmd

BF16 = mybir.dt.bfloat16
F32 = mybir.dt.float32
AF = mybir.ActivationFunctionType

B, T, I, H, O = 256, 512, 256, 512, 3
NCORES = 8
BL = B // NCORES          # 32 batch rows per core
SC = 8                    # time steps per chunk
CW = SC * BL              # 256 cols per chunk
NCH = T // SC             # 64 chunks
TOT = T * BL              # 16384 cols total
SLACK = 2 * CW            # x prefetch overrun slack (cols)
NMAC = NCH + 4            # macro-steps incl pipeline drain (l2 lags 4)

# gate blocks in psum-slot order: i | f | o | g  (slot = blk*4 + j)
GATE_BASE = [0, 512, 1536, 1024]   # i, f, o, g row bases in canonical layout
KCS = [2, 4, 4]                    # K chunks per layer (256, 512, 512)


def _slot_row(slot):
    return GATE_BASE[slot // 4] + 128 * (slot % 4)


def _build():
    nc = bacc.Bacc("TRN2", target_bir_lowering=False, debug=False,
                   num_devices=NCORES)

    xt = nc.dram_tensor("x_t", (128, 2, TOT + SLACK), BF16, kind="ExternalInput")
    wih = [nc.dram_tensor(f"wih{l}", (128, KCS[l] * 2048), BF16,
                          kind="ExternalInput") for l in range(3)]
    whh = [nc.dram_tensor(f"whh{l}", (128, 4 * 2048), BF16,
                          kind="ExternalInput") for l in range(3)]
    bias_d = nc.dram_tensor("bias", (128, 48), F32, kind="ExternalInput")
    fcw_d = nc.dram_tensor("fcw", (128, 12), BF16, kind="ExternalInput")
    fcb_d = nc.dram_tensor("fcb", (3, 1), F32, kind="ExternalInput")
    out_d = nc.dram_tensor("out", (3, BL), F32, kind="ExternalOutput")

    with tile.TileContext(nc) as tc, ExitStack() as ctx:
        const = ctx.enter_context(tc.tile_pool(name="const", bufs=1))
        wih_sb = [const.tile([128, KCS[l] * 2048], BF16, tag=f"wih{l}",
                             name=f"wih_sb{l}") for l in range(3)]
        whh_sb = [const.tile([128, 4 * 2048], BF16, tag=f"whh{l}",
                             name=f"whh_sb{l}") for l in range(3)]
        bias_sb = const.tile([128, 48], F32, tag="bias")
        fcw_sb = const.tile([128, 12], BF16, tag="fcw")
        fcb_sb = const.tile([3, 1], F32, tag="fcb")
        for l in range(3):
            nc.sync.dma_start(wih_sb[l][:], wih[l].ap())
            nc.sync.dma_start(whh_sb[l][:], whh[l].ap())
        nc.sync.dma_start(bias_sb[:], bias_d.ap())
        nc.sync.dma_start(fcw_sb[:], fcw_d.ap())
        nc.sync.dma_start(fcb_sb[:], fcb_d.ap())

        big = ctx.enter_context(tc.tile_pool(name="big", bufs=1))
        # G buffers: per layer, 2 chunk-parities x 16 slots x CW cols
        g_buf = [big.tile([128, 2 * 16 * CW], BF16, tag=f"gbuf{l}",
                          name=f"g_buf{l}") for l in range(3)]
        g3 = [g_buf[l][:].rearrange("p (s c) -> p s c", c=CW) for l in range(3)]
        # h sequence: per layer, 2 parities x 4 k-blocks x CW cols
        h_st = [big.tile([128, 2 * 4 * CW], BF16, tag=f"hst{l}",
                         name=f"h_st{l}") for l in range(3)]
        h3 = [h_st[l][:].rearrange("p (x c) -> p x c", c=CW) for l in range(3)]
        # x staging: 4 chunk bufs x 2 k-blocks
        in_buf = big.tile([128, 4 * 2 * CW], BF16, tag="inbuf")
        i3 = in_buf[:].rearrange("p (b c) -> p b c", c=CW)   # [128, 8, CW]
        c_t = [big.tile([128, 128], F32, tag=f"cst{l}", name=f"c_t{l}")
               for l in range(3)]

        ew = ctx.enter_context(tc.tile_pool(name="ew", bufs=2))
        ps_rec = ctx.enter_context(tc.tile_pool(name="psr", bufs=4, space="PSUM"))
        ps_pa = ctx.enter_context(tc.tile_pool(name="psa", bufs=3, space="PSUM"))
        ps_fc = ctx.enter_context(tc.tile_pool(name="psf", bufs=1, space="PSUM"))

        def in_ap(l, q2, ib, idx):
            """phase-A rhs chunk idx (k-block) for layer l's pa-target."""
            if l == 0:
                return i3[:, bass.ds(2 * ib + idx, 1), :].rearrange(
                    "p a c -> p (a c)")
            return h3[l - 1][:, bass.ds(q2 * 4 + idx, 1), :].rearrange(
                "p a c -> p (a c)")

        def phase_a_slot(l, slot, q2, ib):
            """G[slot] of layer l's next chunk: kc matmuls (N=CW) + bias ACT."""
            kc = KCS[l]
            ps = ps_pa.tile([128, CW], F32, tag="pa", name="pa_ps")
            for k in range(kc):
                nc.tensor.matmul(
                    ps[:],
                    lhsT=wih_sb[l][:, k * 2048 + _slot_row(slot):
                                   k * 2048 + _slot_row(slot) + 128],
                    rhs=in_ap(l, q2, ib, k),
                    start=(k == 0), stop=(k == kc - 1),
                )
            nc.scalar.activation(
                g3[l][:, bass.ds(q2 * 16 + slot, 1), :]
                .rearrange("p a c -> p (a c)"),
                ps[:], AF.Identity,
                bias=bias_sb[:, l * 16 + slot: l * 16 + slot + 1])

        def rec_step(l, s, p2, q2, pa_on, ib):
            """One recurrence time step for layer l (chunk parity p2)."""
            ps = ps_rec.tile([128, 512], F32, tag="rec", name="rec_ps")
            nc.vector.tensor_copy(
                ps[:].rearrange("p (a b) -> p a b", b=BL),
                g3[l][:, bass.ds(p2 * 16, 16), s * BL: (s + 1) * BL])
            hp_base = (q2 if s == 0 else p2) * 4
            hp_col = ((SC - 1) if s == 0 else (s - 1)) * BL
            for slot in range(16):
                for k in range(4):
                    nc.tensor.matmul(
                        ps[:, slot * BL:(slot + 1) * BL],
                        lhsT=whh_sb[l][:, k * 2048 + _slot_row(slot):
                                       k * 2048 + _slot_row(slot) + 128],
                        rhs=h3[l][:, bass.ds(hp_base + k, 1),
                                  hp_col:hp_col + BL].rearrange(
                                      "p a c -> p (a c)"),
                        start=False, stop=(k == 3), skip_group_check=True,
                    )
            gt = ew.tile([128, 512], F32, tag=f"gates{l}", name="gt")
            nc.scalar.activation(gt[:, 0:384], ps[:, 0:384], AF.Sigmoid)
            nc.scalar.activation(gt[:, 384:512], ps[:, 384:512], AF.Tanh)
            t1 = ew.tile([128, 128], F32, tag=f"t1_{l}", name="t1")
            t2 = ew.tile([128, 128], F32, tag=f"t2_{l}", name="t2")
            nc.vector.tensor_mul(t1[:], gt[:, 0:128], gt[:, 384:512])    # i*g
            nc.vector.tensor_mul(t2[:], gt[:, 128:256], c_t[l][:])       # f*c
            nc.vector.tensor_add(c_t[l][:], t1[:], t2[:])
            th = ew.tile([128, 128], F32, tag=f"th{l}", name="th")
            nc.scalar.activation(th[:], c_t[l][:], AF.Tanh)
            nc.vector.tensor_mul(
                h3[l][:, bass.ds(p2 * 4, 4), s * BL:(s + 1) * BL],
                gt[:, 256:384].rearrange("p (a b) -> p a b", b=BL),
                th[:].rearrange("p (a b) -> p a b", b=BL))
            if pa_on:
                phase_a_slot(l, 2 * s, q2, ib)
                phase_a_slot(l, 2 * s + 1, q2, ib)

        def init_layer(l):
            nc.vector.memset(c_t[l][:], 0.0)
            nc.vector.memset(h3[l][:, bass.ds(4, 4), (SC - 1) * BL: SC * BL],
                             0.0)

        def macro(m, dma_off):
            """Emit macro-step m. m is a python int ONLY used for activity
            masks; dma_off is the x HBM column offset (int or ScalarValue)
            for prefetching chunk m+2 (None to skip)."""
            p2 = m & 1
            q2 = (m + 1) & 1
            ib = (m + 1) & 3
            if dma_off is not None:
                ld = (m + 2) & 3
                nc.sync.dma_start(
                    i3[:, bass.ds(2 * ld, 2), :],
                    xt.ap()[:, :, bass.ds(dma_off, CW)])
            for s in range(SC):
                for l in range(3):
                    cl = m - 2 * l
                    if 0 <= cl < NCH:
                        # phase A targets layer l's next chunk cl+1; source
                        # (x or layer l-1's h of chunk cl+1) is ready one
                        # macro earlier.  Template m=8..11 emits pa for all
                        # layers, which at real m=63 makes l0 compute a
                        # nonexistent chunk 64 from x slack zeros: harmless.
                        rec_step(l, s, p2, q2, cl + 1 < NCH, ib)
            # bootstrap: G for chunk 0 of layers 1/2 (their rec starts at
            # macro 2l, pa can't ride on a rec_step that doesn't run yet)
            for l in (1, 2):
                if m == 2 * l - 1:
                    for slot in range(16):
                        phase_a_slot(l, slot, q2, ib)

        # ---- prologue: macros 0..7 unrolled (pipeline fill) ----
        nc.sync.dma_start(i3[:, bass.ds(0, 2), :], xt.ap()[:, :, 0:CW])
        nc.sync.dma_start(i3[:, bass.ds(2, 2), :], xt.ap()[:, :, CW:2 * CW])
        init_layer(0)
        for slot in range(16):
            phase_a_slot(0, slot, 0, 0)    # G chunk 0 from i3 buf 0
        for m in range(8):
            if m == 1:
                init_layer(1)
            if m == 3:
                init_layer(2)
            macro(m, (m + 2) * CW)

        # ---- steady state: macros 8..63 via For_i, body of 4 ----
        with tc.For_i(0, 14, 1) as jv:
            for u in range(4):
                macro(8 + u, jv * (4 * CW) + (10 + u) * CW)

        # ---- epilogue: macros 64..67 ----
        for m in range(NCH, NMAC):
            macro(m, None)

        # final linear head: out.T [3, BL] = fcW @ h_last (+ fcB)
        hb = ((NCH - 1) & 1) * 4
        ps = ps_fc.tile([3, BL], F32, tag="fc")
        for k in range(4):
            nc.tensor.matmul(
                ps[:], lhsT=fcw_sb[:, k * 3:(k + 1) * 3],
                rhs=h3[2][:, bass.ds(hb + k, 1), (SC - 1) * BL: SC * BL]
                .rearrange("p a c -> p (a c)"),
                start=(k == 0), stop=(k == 3))
        ob = ew.tile([3, BL], F32, tag="out")
        nc.scalar.activation(ob[:], ps[:], AF.Identity, bias=fcb_sb[:])
        nc.sync.dma_start(out_d.ap(), ob[:])

    nc.compile()
    return nc


def _prep(inputs):
    """Host-side layout prep. Returns per-core in_maps."""
    bf = ml_dtypes.bfloat16
    x = np.asarray(inputs["x"], np.float32)
    wihs = [np.asarray(inputs[f"Wih{l}"], np.float32) for l in range(3)]
    whhs = [np.asarray(inputs[f"Whh{l}"], np.float32) for l in range(3)]

    def wt_pack(w, kcs):  # [2048, K] -> [128, kcs*2048]
        return np.ascontiguousarray(
            w.T.reshape(kcs, 128, 2048).transpose(1, 0, 2)
            .reshape(128, kcs * 2048)).astype(bf)

    shared = {}
    for l in range(3):
        shared[f"wih{l}"] = wt_pack(wihs[l], KCS[l])
        shared[f"whh{l}"] = wt_pack(whhs[l], 4)
    bias = np.zeros((128, 48), np.float32)
    for l in range(3):
        bl_ = (np.asarray(inputs[f"bih{l}"], np.float32)
               + np.asarray(inputs[f"bhh{l}"], np.float32))
        for slot in range(16):
            r = _slot_row(slot)
            bias[:, l * 16 + slot] = bl_[r:r + 128]
    shared["bias"] = bias
    shared["fcw"] = np.ascontiguousarray(
        np.asarray(inputs["fcW"], np.float32).T.reshape(4, 128, 3)
        .transpose(1, 0, 2).reshape(128, 12)).astype(bf)
    shared["fcb"] = np.asarray(inputs["fcB"], np.float32).reshape(3, 1)

    in_maps = []
    for c in range(NCORES):
        xc = x[c * BL:(c + 1) * BL]                       # [32, 512, 256]
        xp = xc.transpose(2, 1, 0).reshape(2, 128, TOT)   # [2,128,16384]
        xp = np.ascontiguousarray(xp.transpose(1, 0, 2))  # [128,2,16384]
        xp = np.concatenate(
            [xp, np.zeros((128, 2, SLACK), np.float32)], axis=2).astype(bf)
        in_maps.append({"x_t": xp, **shared})
    return in_maps


_NC_CACHE = None


def kernel(**inputs):
    global _NC_CACHE
    if _NC_CACHE is None:
        _NC_CACHE = _build()
    nc = _NC_CACHE
    in_maps = _prep(inputs)
    res = run_bass_kernel_spmd(nc, in_maps, core_ids=list(range(NCORES)))
    out = np.empty((B, O), np.float32)
    for c in range(NCORES):
        out[c * BL:(c + 1) * BL] = res.results[c]["out"].T
    return out


# revision 10
# speedup vs baseline: 9.1326x; 1.0135x over previous
"""3-layer LSTM (B=256,T=512,I=256,H=512) + linear head on 8 NeuronCores.

V4: data-parallel over batch (32/core), gate-major weight-stationary
recurrence (64 LDW+MM pairs per layer-step at ~27ns/pair warm), with the
THREE LAYERS' recurrences interleaved in a chunk-lagged software pipeline
(layer l runs chunk c-2l) so each layer's activation/elementwise tail is
hidden under the other layers' matmul streams.  Inter-layer h stays in
SBUF (no DRAM roundtrip).  All SBUF addressing is static: the For_i body
unrolls 4 macro-steps (only DMA HBM offsets use the loop variable), which
keeps the Tensor sequencer free of per-instruction address regops.

Per layer-step: PSUM [128,512] = 16 gate slots x 32 batch (slot order
i|f|o|g), preloaded with G = Wih@x(+bias) via one DVE cast, accumulated
by 64 weight-stationary bf16 matmuls (K=128,M=128,N=32), evacuated by 2
ACTs (sigmoid|tanh); c stays f32-resident; h is written bf16 into the
layout both the next matmul and the next layer's phase-A consume.
"""

import numpy as np
import ml_dtypes
from contextlib import ExitStack

import concourse.bass as bass
import concourse.bacc as bacc
import concourse.tile as tile
from concourse import mybir
from concourse.bass_utils import run_bass_kernel_spmd

BF16 = mybir.dt.bfloat16
F32 = mybir.dt.float32
AF = mybir.ActivationFunctionType

B, T, I, H, O = 256, 512, 256, 512, 3
NCORES = 8
BL = B // NCORES          # 32 batch rows per core
SC = 8                    # time steps per chunk
CW = SC * BL              # 256 cols per chunk
NCH = T // SC             # 64 chunks
TOT = T * BL              # 16384 cols total
SLACK = 2 * CW            # x prefetch overrun slack (cols)
NMAC = NCH + 4            # macro-steps incl pipeline drain (l2 lags 4)

# gate blocks in psum-slot order: i | f | o | g  (slot = blk*4 + j)
GATE_BASE = [0, 512, 1536, 1024]   # i, f, o, g row bases in canonical layout
KCS = [2, 4, 4]                    # K chunks per layer (256, 512, 512)


def _slot_row(slot):
    return GATE_BASE[slot // 4] + 128 * (slot % 4)


def _build():
    nc = bacc.Bacc("TRN2", target_bir_lowering=False, debug=False,
                   num_devices=NCORES)

    xt = nc.dram_tensor("x_t", (128, 2, TOT + SLACK), BF16, kind="ExternalInput")
    wih = [nc.dram_tensor(f"wih{l}", (128, KCS[l] * 2048), BF16,
                          kind="ExternalInput") for l in range(3)]
    whh = [nc.dram_tensor(f"whh{l}", (128, 4 * 2048), BF16,
                          kind="ExternalInput") for l in range(3)]
    bias_d = nc.dram_tensor("bias", (128, 48), F32, kind="ExternalInput")
    fcw_d = nc.dram_tensor("fcw", (128, 12), BF16, kind="ExternalInput")
    fcb_d = nc.dram_tensor("fcb", (3, 1), F32, kind="ExternalInput")
    out_d = nc.dram_tensor("out", (3, BL), F32, kind="ExternalOutput")

    with tile.TileContext(nc) as tc, ExitStack() as ctx:
        const = ctx.enter_context(tc.tile_pool(name="const", bufs=1))
        wih_sb = [const.tile([128, KCS[l] * 2048], BF16, tag=f"wih{l}",
                             name=f"wih_sb{l}") for l in range(3)]
        whh_sb = [const.tile([128, 4 * 2048], BF16, tag=f"whh{l}",
                             name=f"whh_sb{l}") for l in range(3)]
        bias_sb = const.tile([128, 48], F32, tag="bias")
        fcw_sb = const.tile([128, 12], BF16, tag="fcw")
        fcb_sb = const.tile([3, 1], F32, tag="fcb")
        for l in range(3):
            nc.sync.dma_start(wih_sb[l][:], wih[l].ap())
            nc.sync.dma_start(whh_sb[l][:], whh[l].ap())
        nc.sync.dma_start(bias_sb[:], bias_d.ap())
        nc.sync.dma_start(fcw_sb[:], fcw_d.ap())
        nc.sync.dma_start(fcb_sb[:], fcb_d.ap())

        big = ctx.enter_context(tc.tile_pool(name="big", bufs=1))
        # G buffers: per layer, 2 chunk-parities x 16 slots x CW cols
        g_buf = [big.tile([128, 2 * 16 * CW], BF16, tag=f"gbuf{l}",
                          name=f"g_buf{l}") for l in range(3)]
        g3 = [g_buf[l][:].rearrange("p (s c) -> p s c", c=CW) for l in range(3)]
        # h sequence: per layer, 2 parities x 4 k-blocks x CW cols
        h_st = [big.tile([128, 2 * 4 * CW], BF16, tag=f"hst{l}",
                         name=f"h_st{l}") for l in range(3)]
        h3 = [h_st[l][:].rearrange("p (x c) -> p x c", c=CW) for l in range(3)]
        # x staging: 4 chunk bufs x 2 k-blocks
        in_buf = big.tile([128, 4 * 2 * CW], BF16, tag="inbuf")
        i3 = in_buf[:].rearrange("p (b c) -> p b c", c=CW)   # [128, 8, CW]
        c_t = [big.tile([128, 128], F32, tag=f"cst{l}", name=f"c_t{l}")
               for l in range(3)]

        ew = ctx.enter_context(tc.tile_pool(name="ew", bufs=2))
        ps_rec = ctx.enter_context(tc.tile_pool(name="psr", bufs=4, space="PSUM"))
        ps_pa = ctx.enter_context(tc.tile_pool(name="psa", bufs=3, space="PSUM"))
        ps_fc = ctx.enter_context(tc.tile_pool(name="psf", bufs=1, space="PSUM"))

        def in_ap(l, q2, ib, idx):
            """phase-A rhs chunk idx (k-block) for layer l's pa-target."""
            if l == 0:
                return i3[:, bass.ds(2 * ib + idx, 1), :].rearrange(
                    "p a c -> p (a c)")
            return h3[l - 1][:, bass.ds(q2 * 4 + idx, 1), :].rearrange(
                "p a c -> p (a c)")

        def phase_a_slot(l, slot, q2, ib):
            """G[slot] of layer l's next chunk: kc matmuls (N=CW) + bias ACT."""
            kc = KCS[l]
            ps = ps_pa.tile([128, CW], F32, tag="pa", name="pa_ps")
            for k in range(kc):
                nc.tensor.matmul(
                    ps[:],
                    lhsT=wih_sb[l][:, k * 2048 + _slot_row(slot):
                                   k * 2048 + _slot_row(slot) + 128],
                    rhs=in_ap(l, q2, ib, k),
                    start=(k == 0), stop=(k == kc - 1),
                )
            nc.scalar.activation(
                g3[l][:, bass.ds(q2 * 16 + slot, 1), :]
                .rearrange("p a c -> p (a c)"),
                ps[:], AF.Identity,
                bias=bias_sb[:, l * 16 + slot: l * 16 + slot + 1])

        def rec_step(l, s, p2, q2, pa_on, ib):
            """One recurrence time step for layer l (chunk parity p2)."""
            ps = ps_rec.tile([128, 512], F32, tag="rec", name="rec_ps")
            nc.vector.tensor_copy(
                ps[:].rearrange("p (a b) -> p a b", b=BL),
                g3[l][:, bass.ds(p2 * 16, 16), s * BL: (s + 1) * BL])
            hp_base = (q2 if s == 0 else p2) * 4
            hp_col = ((SC - 1) if s == 0 else (s - 1)) * BL
            for slot in range(16):
                for k in range(4):
                    nc.tensor.matmul(
                        ps[:, slot * BL:(slot + 1) * BL],
                        lhsT=whh_sb[l][:, k * 2048 + _slot_row(slot):
                                       k * 2048 + _slot_row(slot) + 128],
                        rhs=h3[l][:, bass.ds(hp_base + k, 1),
                                  hp_col:hp_col + BL].rearrange(
                                      "p a c -> p (a c)"),
                        start=False, stop=(k == 3), skip_group_check=True,
                    )
            gt = ew.tile([128, 512], F32, tag=f"gates{l}", name="gt")
            nc.scalar.activation(gt[:, 0:384], ps[:, 0:384], AF.Sigmoid)
            nc.scalar.activation(gt[:, 384:512], ps[:, 384:512], AF.Tanh)
            t1 = ew.tile([128, 128], F32, tag=f"t1_{l}", name="t1")
            t2 = ew.tile([128, 128], F32, tag=f"t2_{l}", name="t2")
            nc.vector.tensor_mul(t1[:], gt[:, 0:128], gt[:, 384:512])    # i*g
            nc.vector.tensor_mul(t2[:], gt[:, 128:256], c_t[l][:])       # f*c
            nc.vector.tensor_add(c_t[l][:], t1[:], t2[:])
            th = ew.tile([128, 128], F32, tag=f"th{l}", name="th")
            nc.scalar.activation(th[:], c_t[l][:], AF.Tanh)
            nc.vector.tensor_mul(
                h3[l][:, bass.ds(p2 * 4, 4), s * BL:(s + 1) * BL],
                gt[:, 256:384].rearrange("p (a b) -> p a b", b=BL),
                th[:].rearrange("p (a b) -> p a b", b=BL))
            if pa_on:
                phase_a_slot(l, 2 * s, q2, ib)
                phase_a_slot(l, 2 * s + 1, q2, ib)

        def init_layer(l):
            nc.vector.memset(c_t[l][:], 0.0)
            nc.vector.memset(h3[l][:, bass.ds(4, 4), (SC - 1) * BL: SC * BL],
                             0.0)

        def macro(m, dma_off):
            """Emit macro-step m. m is a python int ONLY used for activity
            masks; dma_off is the x HBM column offset (int or ScalarValue)
            for prefetching chunk m+2 (None to skip)."""
            p2 = m & 1
            q2 = (m + 1) & 1
            ib = (m + 1) & 3
            if dma_off is not None:
                ld = (m + 2) & 3
                nc.sync.dma_start(
                    i3[:, bass.ds(2 * ld, 2), :],
                    xt.ap()[:, :, bass.ds(dma_off, CW)])
            for s in range(SC):
                for l in range(3):
                    cl = m - 2 * l
                    if 0 <= cl < NCH:
                        # phase A targets layer l's next chunk cl+1; source
                        # (x or layer l-1's h of chunk cl+1) is ready one
                        # macro earlier.  Template m=8..11 emits pa for all
                        # layers, which at real m=63 makes l0 compute a
                        # nonexistent chunk 64 from x slack zeros: harmless.
                        rec_step(l, s, p2, q2, cl + 1 < NCH, ib)
            # bootstrap: G for chunk 0 of layers 1/2 (their rec starts at
            # macro 2l, pa can't ride on a rec_step that doesn't run yet)
            for l in (1, 2):
                if m == 2 * l - 1:
                    for slot in range(16):
                        phase_a_slot(l, slot, q2, ib)

        # ---- prologue: macros 0..7 unrolled (pipeline fill) ----
        nc.sync.dma_start(i3[:, bass.ds(0, 2), :], xt.ap()[:, :, 0:CW])
        nc.sync.dma_start(i3[:, bass.ds(2, 2), :], xt.ap()[:, :, CW:2 * CW])
        init_layer(0)
        for slot in range(16):
            phase_a_slot(0, slot, 0, 0)    # G chunk 0 from i3 buf 0
        for m in range(8):
            if m == 1:
                init_layer(1)
            if m == 3:
                init_layer(2)
            macro(m, (m + 2) * CW)

        # ---- steady state: macros 8..63 via For_i, body of 8 ----
        with tc.For_i(0, 7, 1) as jv:
            for u in range(8):
                macro(8 + u, jv * (8 * CW) + (10 + u) * CW)

        # ---- epilogue: macros 64..67 ----
        for m in range(NCH, NMAC):
            macro(m, None)

        # final linear head: out.T [3, BL] = fcW @ h_last (+ fcB)
        hb = ((NCH - 1) & 1) * 4
        ps = ps_fc.tile([3, BL], F32, tag="fc")
        for k in range(4):
            nc.tensor.matmul(
                ps[:], lhsT=fcw_sb[:, k * 3:(k + 1) * 3],
                rhs=h3[2][:, bass.ds(hb + k, 1), (SC - 1) * BL: SC * BL]
                .rearrange("p a c -> p (a c)"),
                start=(k == 0), stop=(k == 3))
        ob = ew.tile([3, BL], F32, tag="out")
        nc.scalar.activation(ob[:], ps[:], AF.Identity, bias=fcb_sb[:])
        nc.sync.dma_start(out_d.ap(), ob[:])

    nc.compile()
    return nc


def _prep(inputs):
    """Host-side layout prep. Returns per-core in_maps."""
    bf = ml_dtypes.bfloat16
    x = np.asarray(inputs["x"], np.float32)
    wihs = [np.asarray(inputs[f"Wih{l}"], np.float32) for l in range(3)]
    whhs = [np.asarray(inputs[f"Whh{l}"], np.float32) for l in range(3)]

    def wt_pack(w, kcs):  # [2048, K] -> [128, kcs*2048]
        return np.ascontiguousarray(
            w.T.reshape(kcs, 128, 2048).transpose(1, 0, 2)
            .reshape(128, kcs * 2048)).astype(bf)

    shared = {}
    for l in range(3):
        shared[f"wih{l}"] = wt_pack(wihs[l], KCS[l])
        shared[f"whh{l}"] = wt_pack(whhs[l], 4)
    bias = np.zeros((128, 48), np.float32)
    for l in range(3):
        bl_ = (np.asarray(inputs[f"bih{l}"], np.float32)
               + np.asarray(inputs[f"bhh{l}"], np.float32))
        for slot in range(16):
            r = _slot_row(slot)
            bias[:, l * 16 + slot] = bl_[r:r + 128]
    shared["bias"] = bias
    shared["fcw"] = np.ascontiguousarray(
        np.asarray(inputs["fcW"], np.float32).T.reshape(4, 128, 3)
        .transpose(1, 0, 2).reshape(128, 12)).astype(bf)
    shared["fcb"] = np.asarray(inputs["fcB"], np.float32).reshape(3, 1)

    in_maps = []
    for c in range(NCORES):
        xc = x[c * BL:(c + 1) * BL]                       # [32, 512, 256]
        xp = xc.transpose(2, 1, 0).reshape(2, 128, TOT)   # [2,128,16384]
        xp = np.ascontiguousarray(xp.transpose(1, 0, 2))  # [128,2,16384]
        xp = np.concatenate(
            [xp, np.zeros((128, 2, SLACK), np.float32)], axis=2).astype(bf)
        in_maps.append({"x_t": xp, **shared})
    return in_maps


_NC_CACHE = None


def kernel(**inputs):
    global _NC_CACHE
    if _NC_CACHE is None:
        _NC_CACHE = _build()
    nc = _NC_CACHE
    in_maps = _prep(inputs)
    res = run_bass_kernel_spmd(nc, in_maps, core_ids=list(range(NCORES)))
    out = np.empty((B, O), np.float32)
    for c in range(NCORES):
        out[c * BL:(c + 1) * BL] = res.results[c]["out"].T
    return out
